# revision 8
# baseline (speedup 1.0000x reference)
"""Trainium2 Bass kernel for nn_CausalSelfAttention (GQA + partial RoPE + qk-norm
+ value-embedding gating + output gating).

Sharding (8 cores): core c = b*4 + m handles batch b (of 2) and kv-head m (of 4),
i.e. q-heads 4m..4m+3  (tensor-parallel over heads x data-parallel over batch,
per the P('data','model') hint).  The o-projection contraction runs after an
AllToAll that redistributes attention outputs from head-sharded to
sequence-sharded layout; core (b, m) then produces output rows
[512*m, 512*m+512) of batch b.
"""

import os
import sys

import numpy as np

sys.path.insert(0, "/opt/trn_rl_repo")

from contextlib import ExitStack

import concourse.bass as bass
import concourse.mybir as mybir
import concourse.tile as tile
from concourse import bacc, library_config
from concourse.bass import IndirectOffsetOnAxis
from concourse.masks import make_identity

F32 = mybir.dt.float32
BF16 = mybir.dt.bfloat16
I32 = mybir.dt.int32
AX = mybir.AxisListType
ALU = mybir.AluOpType
ACTF = mybir.ActivationFunctionType

B, S, D = 2, 2048, 1024
N, M, HD = 16, 4, 64
G = N // M            # q heads per kv head = 4
V = 32000
GIN = 12
ROT, HALF = 32, 16
EPS = 1e-6
ST = S // 128          # 16 S-tiles
SCALE = 1.0 / 8.0      # 1/sqrt(HD)

NQT = S // 256         # 8 q-tiles of 256


def build_nc(replica_groups=None, debug=False):
    if replica_groups is None:
        replica_groups = [[0, 1, 2, 3, 4, 5, 6, 7]]
    nc = bacc.Bacc(None, target_bir_lowering=False, debug=debug)

    xT = nc.declare_dram_parameter("xT", [D, S], F32, isOutput=False)
    wq = nc.declare_dram_parameter("wq", [D, G * HD], F32, isOutput=False)
    kvg = nc.declare_dram_parameter("kvg", [D, 2 * HD + 5], F32, isOutput=False)
    wo = nc.declare_dram_parameter("wo", [2 * D, D], F32, isOutput=False)
    lam = nc.declare_dram_parameter("lam", [1, 2], F32, isOutput=False)
    vemb = nc.declare_dram_parameter("vemb", [V, HD], F32, isOutput=False)
    tok = nc.declare_dram_parameter("tok", [128, ST], I32, isOutput=False)
    trig = nc.declare_dram_parameter("trig", [128, ST * 2 * HALF], F32, isOutput=False)
    maskd = nc.declare_dram_parameter("maskd", [128, S], F32, isOutput=False)
    out = nc.declare_dram_parameter("out", [512, D], F32, isOutput=True)

    with tile.TileContext(nc) as tc, ExitStack() as ctx:
        nc.gpsimd.load_library(library_config.proxy)

        const = ctx.enter_context(tc.tile_pool(name="const", bufs=1))
        work = ctx.enter_context(tc.tile_pool(name="work", bufs=3))
        small = ctx.enter_context(tc.tile_pool(name="small", bufs=4))
        ptp = ctx.enter_context(tc.tile_pool(name="ptp", bufs=3))
        ep = ctx.enter_context(tc.tile_pool(name="ep", bufs=3))
        psm = ctx.enter_context(tc.tile_pool(name="psm", bufs=4, space="PSUM"))
        psa = ctx.enter_context(tc.tile_pool(name="psa", bufs=2, space="PSUM"))
        dram = ctx.enter_context(tc.tile_pool(name="dram", bufs=1, space="DRAM"))

        # ---------------- persistent SBUF ----------------
        xT_sb = const.tile([128, 8, S], BF16)
        wq_sb = const.tile([128, 8, G * HD], BF16)
        kvg_sb = const.tile([128, 8, 133], BF16)
        wo_sb = const.tile([128, 16, D], BF16)
        trig_sb = const.tile([128, ST, 2 * HALF], F32)
        maskd_sb = const.tile([128, S], BF16)
        tok_sb = const.tile([128, ST], I32)
        lam_sb = const.tile([1, 2], F32)
        vl_bc = const.tile([128, 1], F32)
        l2_bc = const.tile([128, 1], F32)
        l2_sb = const.tile([1, 1], F32)
        id_bf = const.tile([128, 128], BF16)
        id_f32 = const.tile([128, 128], F32)
        qT_sb = const.tile([64, G, S], BF16)          # q^T per head: [hd, q]
        kT_sb = const.tile([64, S], BF16)             # k^T: [hd, keys]
        vaug_sb = const.tile([128, ST, HD + 1], BF16)  # v rows + 0.5-column
        g_sb = const.tile([128, ST, 1], F32)           # sigmoid(ve_g)
        agS_sb = const.tile([1, G, S], F32)            # sigmoid(ag) [1, head, q]
        agin_sb = const.tile([128, 2, S], BF16)        # attn^T staging (4 heads)
        oT_sb = const.tile([128, 16, 512], BF16)       # o-proj lhsT after A2A

        # ---------------- loads ----------------
        nc.gpsimd.dma_start(out=xT_sb[:], in_=xT.rearrange("(t p) s -> p t s", p=128))
        nc.gpsimd.dma_start(out=wq_sb[:], in_=wq.rearrange("(t p) c -> p t c", p=128))
        nc.gpsimd.dma_start(out=kvg_sb[:], in_=kvg.rearrange("(t p) c -> p t c", p=128))
        nc.gpsimd.dma_start(out=wo_sb[:], in_=wo.rearrange("(t p) c -> p t c", p=128))
        nc.sync.dma_start(out=trig_sb[:], in_=trig.rearrange("p (t c) -> p t c", t=ST))
        nc.gpsimd.dma_start(out=maskd_sb[:], in_=maskd[:, :])
        nc.sync.dma_start(out=tok_sb[:], in_=tok[:, :])
        nc.sync.dma_start(out=lam_sb[:], in_=lam[:, :])

        eps_sb = const.tile([128, 1], F32)
        nc.vector.memset(eps_sb[:], EPS)
        nc.gpsimd.partition_broadcast(vl_bc[:], lam_sb[0:1, 0:1])
        nc.scalar.mul(l2_sb[:], lam_sb[0:1, 1:2], 2.0)
        nc.gpsimd.partition_broadcast(l2_bc[:], l2_sb[0:1, 0:1])
        make_identity(nc, id_bf[:])
        make_identity(nc, id_f32[:])
        nc.vector.memset(vaug_sb[:, :, HD : HD + 1], 0.5)

        # ---------------- phase 1: qkv projections, norm, rope, mix ----------
        for s in range(ST):
            q_ps = psm.tile([128, G * HD], F32, tag="mm")
            kv_ps = psm.tile([128, 133], F32, tag="mm")
            for d in range(8):
                lhs = xT_sb[:, d, 128 * s : 128 * s + 128]
                nc.tensor.matmul(q_ps[:], lhs, wq_sb[:, d, :],
                                 start=(d == 0), stop=(d == 7))
                nc.tensor.matmul(kv_ps[:], lhs, kvg_sb[:, d, :],
                                 start=(d == 0), stop=(d == 7))

            # --- gates: sigmoid(z) for [ve_g, ag*4] ---
            nc.scalar.activation(g_sb[:, s, :], kv_ps[:, 128:129], ACTF.Sigmoid)

            # --- value-embedding gather + mix -> vaug ---
            ve_t = work.tile([128, HD], F32, tag="ve")
            nc.gpsimd.indirect_dma_start(
                out=ve_t[:], out_offset=None, in_=vemb[:, :],
                in_offset=IndirectOffsetOnAxis(ap=tok_sb[:, s : s + 1], axis=0))
            gsc = small.tile([128, 1], F32, tag="gsc")
            nc.vector.tensor_mul(gsc[:], g_sb[:, s, 0:1], l2_bc[:])
            vmix = work.tile([128, HD], F32, tag="vmix")
            nc.vector.tensor_scalar_mul(vmix[:], kv_ps[:, 64:128], vl_bc[:, 0:1])
            nc.vector.scalar_tensor_tensor(
                out=vaug_sb[:, s, 0:HD], in0=ve_t[:], scalar=gsc[:, 0:1],
                in1=vmix[:], op0=ALU.mult, op1=ALU.add)

            # --- k: qk-norm + rope -> kT ---
            k2 = work.tile([128, HD], F32, tag="k2")
            nc.scalar.activation(k2[:], kv_ps[:, 0:64], ACTF.Square)
            msk = small.tile([128, 1], F32, tag="msk")
            nc.vector.tensor_reduce(msk[:], k2[:], AX.X, ALU.add)
            nc.scalar.activation(msk[:], msk[:], ACTF.Sqrt, bias=eps_sb[:, 0:1], scale=1.0 / HD)
            rk = small.tile([128, 1], F32, tag="rk")
            nc.vector.reciprocal(rk[:], msk[:])
            kn = work.tile([128, HD], F32, tag="kn")
            nc.vector.tensor_scalar_mul(kn[:], kv_ps[:, 0:64], rk[:, 0:1])
            kf = work.tile([128, HD], BF16, tag="kf")
            cos = trig_sb[:, s, 0:HALF]
            sin = trig_sb[:, s, HALF : 2 * HALF]
            t1 = work.tile([128, HALF], F32, tag="t1")
            t2 = work.tile([128, HALF], F32, tag="t2")
            nc.vector.tensor_mul(t1[:], kn[:, 0:HALF], cos)
            nc.vector.scalar_tensor_tensor(out=t2[:], in0=kn[:, HALF:ROT],
                                           scalar=-1.0, in1=sin,
                                           op0=ALU.mult, op1=ALU.mult)
            nc.vector.tensor_add(kf[:, 0:HALF], t1[:], t2[:])
            nc.vector.tensor_mul(t1[:], kn[:, 0:HALF], sin)
            nc.vector.tensor_mul(t2[:], kn[:, HALF:ROT], cos)
            nc.vector.tensor_add(kf[:, HALF:ROT], t1[:], t2[:])
            nc.vector.tensor_copy(kf[:, ROT:HD], kn[:, ROT:HD])
            tr_ps = psm.tile([128, 128], BF16, tag="mm")
            nc.tensor.transpose(tr_ps[0:64, :], kf[:], id_bf[:])
            nc.vector.tensor_copy(kT_sb[:, 128 * s : 128 * s + 128], tr_ps[0:64, :])

            # --- q: qk-norm + rope (4 heads at once via 3D views) ---
            q2 = work.tile([128, G * HD], F32, tag="q2")
            nc.scalar.activation(q2[:], q_ps[:], ACTF.Square)
            msq = small.tile([128, G], F32, tag="msq")
            nc.vector.tensor_reduce(
                msq[:], q2[:].rearrange("p (g d) -> p g d", g=G), AX.X, ALU.add)
            nc.scalar.activation(msq[:], msq[:], ACTF.Sqrt, bias=eps_sb[:, 0:1], scale=1.0 / HD)
            rq = small.tile([128, G], F32, tag="rq")
            nc.vector.reciprocal(rq[:], msq[:])
            qn = work.tile([128, G, HD], F32, tag="qn")
            rq_bc = rq[:].rearrange("p (g o) -> p g o", o=1).to_broadcast([128, G, HD])
            nc.vector.tensor_mul(qn[:], q_ps[:].rearrange("p (g d) -> p g d", g=G), rq_bc)
            qf = work.tile([128, G, HD], BF16, tag="qf")
            cos_bc = cos.rearrange("p (o f) -> p o f", o=1).to_broadcast([128, G, HALF])
            sin_bc = sin.rearrange("p (o f) -> p o f", o=1).to_broadcast([128, G, HALF])
            t3 = work.tile([128, G, HALF], F32, tag="t3")
            t4 = work.tile([128, G, HALF], F32, tag="t4")
            nc.vector.tensor_mul(t3[:], qn[:, :, 0:HALF], cos_bc)
            nc.vector.scalar_tensor_tensor(out=t4[:], in0=qn[:, :, HALF:ROT],
                                           scalar=-1.0, in1=sin_bc,
                                           op0=ALU.mult, op1=ALU.mult)
            nc.vector.tensor_add(qf[:, :, 0:HALF], t3[:], t4[:])
            nc.vector.tensor_mul(t3[:], qn[:, :, 0:HALF], sin_bc)
            nc.vector.tensor_mul(t4[:], qn[:, :, HALF:ROT], cos_bc)
            nc.vector.tensor_add(qf[:, :, HALF:ROT], t3[:], t4[:])
            nc.vector.tensor_copy(qf[:, :, ROT:HD], qn[:, :, ROT:HD])
            for blk in range(2):
                trq = psm.tile([128, 128], BF16, tag="mm")
                nc.tensor.transpose(
                    trq[:], qf[:].rearrange("p g d -> p (g d)")[:, 128 * blk : 128 * blk + 128],
                    id_bf[:])
                nc.vector.tensor_copy(
                    qT_sb[:, 2 * blk, 128 * s : 128 * s + 128], trq[0:64, :])
                nc.vector.tensor_copy(
                    qT_sb[:, 2 * blk + 1, 128 * s : 128 * s + 128], trq[64:128, :])


        # ----- a_g^T via matmul (gate col as lhsT), sigmoid on partition 0 ---
        psz = ctx.enter_context(tc.tile_pool(name="psz", bufs=1, space="PSUM"))
        for h in range(G):
            for half in range(2):
                agz_ps = psz.tile([1, 1024], F32, tag="agz")
                for blk in range(2):
                    nc.tensor.matmul(
                        agz_ps[0:1, 512 * blk : 512 * blk + 512],
                        kvg_sb[:, 0, 129 + h : 130 + h],
                        xT_sb[:, 0, 1024 * half + 512 * blk : 1024 * half + 512 * blk + 512],
                        start=True, stop=True)
                nc.scalar.activation(agS_sb[0:1, h, 1024 * half : 1024 * half + 1024],
                                     agz_ps[:], ACTF.Sigmoid)

        # ---------------- phase 2: attention ----------------
        for c in range(NQT):
            for h in range(G):
                at_ps = psa.tile([128, 256], F32, tag="acc")
                nkt = 2 * c + 2
                for kt in range(nkt):
                    sc_ps = psm.tile([128, 256], F32, tag="mm")
                    nc.tensor.matmul(
                        sc_ps[:], kT_sb[:, 128 * kt : 128 * kt + 128],
                        qT_sb[:, h, 256 * c : 256 * c + 256],
                        start=True, stop=True)
                    pT = ptp.tile([128, 256], BF16, tag="pT")
                    nc.scalar.activation(pT[:], sc_ps[:], ACTF.Exp, scale=SCALE)
                    if kt == 2 * c:
                        nc.vector.tensor_mul(
                            pT[:, 0:128], pT[:, 0:128],
                            maskd_sb[:, 128 * kt : 128 * kt + 128])
                    elif kt == 2 * c + 1:
                        nc.vector.memset(pT[:, 0:128], 0.0)
                        nc.vector.tensor_mul(
                            pT[:, 128:256], pT[:, 128:256],
                            maskd_sb[:, 128 * kt : 128 * kt + 128])
                    nc.tensor.matmul(
                        at_ps[0:65, :], vaug_sb[:, kt, :], pT[:],
                        start=(kt == 0), stop=(kt == nkt - 1))

                # epilogue: s_vec = 2*sigmoid(ag)/Z broadcast over dims
                rz = ep.tile([1, 256], F32, tag="rz")
                nc.vector.reciprocal(rz[:], at_ps[64:65, :])
                sv = ep.tile([1, 256], F32, tag="sv")
                nc.vector.tensor_mul(sv[:], rz[:],
                                     agS_sb[0:1, h, 256 * c : 256 * c + 256])
                bc = ep.tile([64, 256], F32, tag="bc")
                nc.gpsimd.partition_broadcast(bc[:], sv[:])
                po = 64 * (h % 2)
                nc.vector.tensor_mul(
                    agin_sb[po : po + 64, h // 2, 256 * c : 256 * c + 256],
                    at_ps[0:64, :], bc[:])

        # ---------------- phase 2.5: AllToAll (8-rank; shards duplicated to
        # both batch halves, receiver selects via zero-padded wo) ----------
        a2a_in = dram.tile([2048, 512], BF16)
        a2a_out = dram.tile([2048, 512], BF16)
        for b2 in range(2):
            for p2 in range(4):
                r0 = 1024 * b2 + 256 * p2
                nc.sync.dma_start(
                    out=a2a_in[r0 : r0 + 256].rearrange("(i p) q -> p i q", p=128),
                    in_=agin_sb[:, :, 512 * p2 : 512 * p2 + 512])
        nc.gpsimd.collective_compute(
            "AllToAll", ALU.bypass, replica_groups=replica_groups,
            ins=[a2a_in[:]], outs=[a2a_out[:]])
        nc.sync.dma_start(out=oT_sb[:],
                          in_=a2a_out[:].rearrange("(t p) q -> p t q", p=128))

        # ---------------- phase 3: o-projection ----------------
        for qs in range(4):
            for hf in range(2):
                op_ps = psa.tile([128, 512], F32, tag="acc")
                for r in range(16):
                    nc.tensor.matmul(
                        op_ps[:], oT_sb[:, r, 128 * qs : 128 * qs + 128],
                        wo_sb[:, r, 512 * hf : 512 * hf + 512],
                        start=(r == 0), stop=(r == 15))
                o_t = work.tile([128, 512], F32, tag="ot")
                nc.vector.tensor_copy(o_t[:], op_ps[:])
                nc.sync.dma_start(
                    out=out[128 * qs : 128 * qs + 128, 512 * hf : 512 * hf + 512],
                    in_=o_t[:])
    nc.finalize()
    return nc


def make_in_maps(x, token_ids, mask, w_q, w_k, w_v, w_o, ve_embed,
                 value_lambda, ve_lambda, ve_gate, attn_gate):
    x = np.asarray(x, np.float32)
    token_ids = np.asarray(token_ids)
    mask = np.asarray(mask)
    w_q = np.asarray(w_q, np.float32)
    w_k = np.asarray(w_k, np.float32)
    w_v = np.asarray(w_v, np.float32)
    w_o = np.asarray(w_o, np.float32)
    ve_embed = np.asarray(ve_embed, np.float32)
    ve_gate = np.asarray(ve_gate, np.float32)
    attn_gate = np.asarray(attn_gate, np.float32)
    lam = np.array([[float(value_lambda), float(ve_lambda)]], np.float32)

    half = ROT // 2
    inv_freq = 1.0 / (1024.0 ** (np.arange(half, dtype=np.float32) / half))
    ang = np.arange(S, dtype=np.float32)[:, None] * inv_freq[None, :]
    cos = np.cos(ang).astype(np.float32)          # [S, 16]
    sin = np.sin(ang).astype(np.float32)
    trig = np.concatenate([cos.reshape(ST, 128, HALF), sin.reshape(ST, 128, HALF)],
                          axis=2)                  # [ST, 128, 32]
    trig = np.ascontiguousarray(trig.transpose(1, 0, 2).reshape(128, ST * 2 * HALF))

    maskd = np.empty((128, S), np.float32)
    for t in range(ST):
        blk = mask[128 * t : 128 * t + 128, 128 * t : 128 * t + 128]
        maskd[:, 128 * t : 128 * t + 128] = blk.T.astype(np.float32)

    wo_pad = np.zeros((2, 2 * D, D), np.float32)
    for b in range(2):
        wo_pad[b, 1024 * b : 1024 * b + 1024] = w_o
    in_maps = []
    for c in range(8):
        b, m = c // 4, c % 4
        tok = np.ascontiguousarray(
            token_ids[b].astype(np.int32).reshape(ST, 128).T)   # [128, ST]
        kvgm = np.zeros((D, 133), np.float32)
        kvgm[:, 0:64] = w_k[:, 64 * m : 64 * m + 64]
        kvgm[:, 64:128] = w_v[:, 64 * m : 64 * m + 64]
        kvgm[0:GIN, 128] = ve_gate[:, m]
        kvgm[0:GIN, 129:133] = attn_gate[:, 4 * m : 4 * m + 4]
        in_maps.append({
            "xT": np.ascontiguousarray(x[b].T),
            "wq": np.ascontiguousarray(w_q[:, 256 * m : 256 * m + 256]),
            "kvg": kvgm,
            "wo": wo_pad[b],
            "lam": lam,
            "vemb": np.ascontiguousarray(ve_embed[:, 64 * m : 64 * m + 64]),
            "tok": tok,
            "trig": trig,
            "maskd": maskd,
        })
    return in_maps


def unshard(results):
    out = np.empty((B, S, D), np.float32)
    for c in range(8):
        b, m = c // 4, c % 4
        out[b, 512 * m : 512 * m + 512, :] = results[c]["out"]
    return out


_NC_CACHE = {}


def kernel(**inputs):
    from concourse.bass_utils import run_bass_kernel_spmd
    if "nc" not in _NC_CACHE:
        _NC_CACHE["nc"] = build_nc()
    nc = _NC_CACHE["nc"]
    in_maps = make_in_maps(**inputs)
    res = run_bass_kernel_spmd(nc, in_maps, core_ids=list(range(8)))
    return unshard(res.results)


if __name__ == "__main__":
    pass


# revision 13
# speedup vs baseline: 1.1082x; 1.1082x over previous
"""Trainium2 Bass kernel for nn_CausalSelfAttention (GQA + partial RoPE + qk-norm
+ value-embedding gating + output gating).

Sharding (8 cores): core c = b*4 + m handles batch b (of 2) and kv-head m (of 4),
i.e. q-heads 4m..4m+3  (tensor-parallel over heads x data-parallel over batch,
per the P('data','model') hint).  The o-projection contraction runs after an
AllToAll that redistributes attention outputs from head-sharded to
sequence-sharded layout; core (b, m) then produces output rows
[512*m, 512*m+512) of batch b.
"""

import os
import sys

import numpy as np

sys.path.insert(0, "/opt/trn_rl_repo")

from contextlib import ExitStack

import concourse.bass as bass
import concourse.mybir as mybir
import concourse.tile as tile
from concourse import bacc, library_config
from concourse.bass import IndirectOffsetOnAxis
from concourse.masks import make_identity

F32 = mybir.dt.float32
BF16 = mybir.dt.bfloat16
I32 = mybir.dt.int32
AX = mybir.AxisListType
ALU = mybir.AluOpType
ACTF = mybir.ActivationFunctionType

B, S, D = 2, 2048, 1024
N, M, HD = 16, 4, 64
G = N // M            # q heads per kv head = 4
V = 32000
GIN = 12
ROT, HALF = 32, 16
EPS = 1e-6
ST = S // 128          # 16 S-tiles
SCALE = 1.0 / 8.0      # 1/sqrt(HD)

NQT = S // 256         # 8 q-tiles of 256


def build_nc(replica_groups=None, debug=False):
    if replica_groups is None:
        replica_groups = [[0, 1, 2, 3, 4, 5, 6, 7]]
    nc = bacc.Bacc(None, target_bir_lowering=False, debug=debug)

    xT = nc.declare_dram_parameter("xT", [D, S], F32, isOutput=False)
    wqkvg = nc.declare_dram_parameter("wqkvg", [D, 389], F32, isOutput=False)
    wo = nc.declare_dram_parameter("wo", [2 * D, D], F32, isOutput=False)
    lam = nc.declare_dram_parameter("lam", [1, 2], F32, isOutput=False)
    vemb = nc.declare_dram_parameter("vemb", [V, HD], F32, isOutput=False)
    tok = nc.declare_dram_parameter("tok", [128, ST], I32, isOutput=False)
    trig = nc.declare_dram_parameter("trig", [128, ST * 2 * HALF], F32, isOutput=False)
    maskd = nc.declare_dram_parameter("maskd", [128, S], F32, isOutput=False)
    out = nc.declare_dram_parameter("out", [512, D], F32, isOutput=True)

    with tile.TileContext(nc) as tc, ExitStack() as ctx:
        nc.gpsimd.load_library(library_config.proxy)

        const = ctx.enter_context(tc.tile_pool(name="const", bufs=1))
        work = ctx.enter_context(tc.tile_pool(name="work", bufs=3))
        small = ctx.enter_context(tc.tile_pool(name="small", bufs=4))
        ptp = ctx.enter_context(tc.tile_pool(name="ptp", bufs=4))
        ep = ctx.enter_context(tc.tile_pool(name="ep", bufs=2))
        psm = ctx.enter_context(tc.tile_pool(name="psm", bufs=4, space="PSUM"))
        psa = ctx.enter_context(tc.tile_pool(name="psa", bufs=3, space="PSUM"))
        dram = ctx.enter_context(tc.tile_pool(name="dram", bufs=1, space="DRAM"))

        # ---------------- persistent SBUF ----------------
        xT_sb = const.tile([128, 8, S], BF16)
        wqkvg_sb = const.tile([128, 8, 389], BF16)
        wo_sb = const.tile([128, 16, D], BF16)
        trig_sb = const.tile([128, ST, 2 * HALF], F32)
        maskd_sb = const.tile([128, S], BF16)
        tok_sb = const.tile([128, ST], I32)
        lam_sb = const.tile([1, 2], F32)
        vl_bc = const.tile([128, 1], F32)
        l2_bc = const.tile([128, 1], F32)
        l2_sb = const.tile([1, 1], F32)
        id_bf = const.tile([128, 128], BF16)
        id_f32 = const.tile([128, 128], F32)
        qT_sb = const.tile([64, G, S], BF16)          # q^T per head: [hd, q]
        kT_sb = const.tile([64, S], BF16)             # k^T: [hd, keys]
        vaug_sb = const.tile([128, ST, HD + 1], BF16)  # v rows + 0.5-column
        g_sb = const.tile([128, ST, 1], F32)           # sigmoid(ve_g)
        agS_sb = const.tile([1, G, S], F32)            # sigmoid(ag) [1, head, q]
        agin_sb = const.tile([128, 2, S], BF16)        # attn^T staging (4 heads)
        oT_sb = const.tile([128, 16, 512], BF16)       # o-proj lhsT after A2A

        # ---------------- loads ----------------
        nc.gpsimd.dma_start(out=xT_sb[:], in_=xT.rearrange("(t p) s -> p t s", p=128))
        nc.gpsimd.dma_start(out=wqkvg_sb[:],
                            in_=wqkvg.rearrange("(t p) c -> p t c", p=128))
        nc.gpsimd.dma_start(out=wo_sb[:], in_=wo.rearrange("(t p) c -> p t c", p=128))
        nc.sync.dma_start(out=trig_sb[:], in_=trig.rearrange("p (t c) -> p t c", t=ST))
        nc.gpsimd.dma_start(out=maskd_sb[:], in_=maskd[:, :])
        nc.sync.dma_start(out=tok_sb[:], in_=tok[:, :])
        nc.sync.dma_start(out=lam_sb[:], in_=lam[:, :])

        eps_sb = const.tile([128, 1], F32)
        nc.vector.memset(eps_sb[:], EPS)
        nc.gpsimd.partition_broadcast(vl_bc[:], lam_sb[0:1, 0:1])
        nc.scalar.mul(l2_sb[:], lam_sb[0:1, 1:2], 2.0)
        nc.gpsimd.partition_broadcast(l2_bc[:], l2_sb[0:1, 0:1])
        make_identity(nc, id_bf[:])
        make_identity(nc, id_f32[:])
        nc.vector.memset(vaug_sb[:, :, HD : HD + 1], 0.5)

        # ---------------- phase 1: qkv projections, norm, rope, mix ----------
        for s in range(ST):
            qkv_ps = psm.tile([128, 389], F32, tag="mm")
            q_ps = qkv_ps[:, 0:256]
            kv_ps = qkv_ps[:, 256:389]
            for d in range(8):
                lhs = xT_sb[:, d, 128 * s : 128 * s + 128]
                nc.tensor.matmul(qkv_ps[:], lhs, wqkvg_sb[:, d, :],
                                 start=(d == 0), stop=(d == 7))

            # --- gates: sigmoid(z) for ve_g ---
            nc.scalar.activation(g_sb[:, s, :], kv_ps[:, 128:129], ACTF.Sigmoid)

            # --- value-embedding gather + mix -> vaug ---
            ve_t = work.tile([128, HD], F32, tag="ve")
            nc.gpsimd.indirect_dma_start(
                out=ve_t[:], out_offset=None, in_=vemb[:, :],
                in_offset=IndirectOffsetOnAxis(ap=tok_sb[:, s : s + 1], axis=0))
            gsc = small.tile([128, 1], F32, tag="gsc")
            nc.vector.tensor_mul(gsc[:], g_sb[:, s, 0:1], l2_bc[:])
            vmix = work.tile([128, HD], F32, tag="vmix")
            nc.vector.tensor_scalar_mul(vmix[:], kv_ps[:, 64:128], vl_bc[:, 0:1])
            nc.vector.scalar_tensor_tensor(
                out=vaug_sb[:, s, 0:HD], in0=ve_t[:], scalar=gsc[:, 0:1],
                in1=vmix[:], op0=ALU.mult, op1=ALU.add)

            # --- k: qk-norm + rope -> kT ---
            k2 = work.tile([128, HD], F32, tag="k2")
            nc.scalar.activation(k2[:], kv_ps[:, 0:64], ACTF.Square)
            msk = small.tile([128, 1], F32, tag="msk")
            nc.vector.tensor_reduce(msk[:], k2[:], AX.X, ALU.add)
            nc.scalar.activation(msk[:], msk[:], ACTF.Sqrt, bias=eps_sb[:, 0:1], scale=1.0 / HD)
            rk = small.tile([128, 1], F32, tag="rk")
            nc.vector.reciprocal(rk[:], msk[:])
            kn = work.tile([128, HD], F32, tag="kn")
            nc.vector.tensor_scalar_mul(kn[:], kv_ps[:, 0:64], rk[:, 0:1])
            kf = work.tile([128, HD], BF16, tag="kf")
            cos = trig_sb[:, s, 0:HALF]
            sin = trig_sb[:, s, HALF : 2 * HALF]
            t1 = work.tile([128, HALF], F32, tag="t1")
            t2 = work.tile([128, HALF], F32, tag="t2")
            nc.vector.tensor_mul(t1[:], kn[:, 0:HALF], cos)
            nc.vector.scalar_tensor_tensor(out=t2[:], in0=kn[:, HALF:ROT],
                                           scalar=-1.0, in1=sin,
                                           op0=ALU.mult, op1=ALU.mult)
            nc.vector.tensor_add(kf[:, 0:HALF], t1[:], t2[:])
            nc.vector.tensor_mul(t1[:], kn[:, 0:HALF], sin)
            nc.vector.tensor_mul(t2[:], kn[:, HALF:ROT], cos)
            nc.vector.tensor_add(kf[:, HALF:ROT], t1[:], t2[:])
            nc.vector.tensor_copy(kf[:, ROT:HD], kn[:, ROT:HD])
            tr_ps = psm.tile([128, 128], BF16, tag="mm")
            nc.tensor.transpose(tr_ps[0:64, :], kf[:], id_bf[:])
            nc.vector.tensor_copy(kT_sb[:, 128 * s : 128 * s + 128], tr_ps[0:64, :])

            # --- q: qk-norm + rope (4 heads at once via 3D views) ---
            q2 = work.tile([128, G * HD], F32, tag="q2")
            nc.scalar.activation(q2[:], q_ps[:], ACTF.Square)
            msq = small.tile([128, G], F32, tag="msq")
            nc.vector.tensor_reduce(
                msq[:], q2[:].rearrange("p (g d) -> p g d", g=G), AX.X, ALU.add)
            nc.scalar.activation(msq[:], msq[:], ACTF.Sqrt, bias=eps_sb[:, 0:1], scale=1.0 / HD)
            rq = small.tile([128, G], F32, tag="rq")
            nc.vector.reciprocal(rq[:], msq[:])
            qn = work.tile([128, G, HD], F32, tag="qn")
            rq_bc = rq[:].rearrange("p (g o) -> p g o", o=1).to_broadcast([128, G, HD])
            nc.vector.tensor_mul(qn[:], q_ps[:].rearrange("p (g d) -> p g d", g=G), rq_bc)
            qf = work.tile([128, G, HD], BF16, tag="qf")
            cos_bc = cos.rearrange("p (o f) -> p o f", o=1).to_broadcast([128, G, HALF])
            sin_bc = sin.rearrange("p (o f) -> p o f", o=1).to_broadcast([128, G, HALF])
            t3 = work.tile([128, G, HALF], F32, tag="t3")
            t4 = work.tile([128, G, HALF], F32, tag="t4")
            nc.vector.tensor_mul(t3[:], qn[:, :, 0:HALF], cos_bc)
            nc.vector.scalar_tensor_tensor(out=t4[:], in0=qn[:, :, HALF:ROT],
                                           scalar=-1.0, in1=sin_bc,
                                           op0=ALU.mult, op1=ALU.mult)
            nc.vector.tensor_add(qf[:, :, 0:HALF], t3[:], t4[:])
            nc.vector.tensor_mul(t3[:], qn[:, :, 0:HALF], sin_bc)
            nc.vector.tensor_mul(t4[:], qn[:, :, HALF:ROT], cos_bc)
            nc.vector.tensor_add(qf[:, :, HALF:ROT], t3[:], t4[:])
            nc.vector.tensor_copy(qf[:, :, ROT:HD], qn[:, :, ROT:HD])
            for blk in range(2):
                trq = psm.tile([128, 128], BF16, tag="mm")
                nc.tensor.transpose(
                    trq[:], qf[:].rearrange("p g d -> p (g d)")[:, 128 * blk : 128 * blk + 128],
                    id_bf[:])
                nc.vector.tensor_copy(
                    qT_sb[:, 2 * blk, 128 * s : 128 * s + 128], trq[0:64, :])
                nc.vector.tensor_copy(
                    qT_sb[:, 2 * blk + 1, 128 * s : 128 * s + 128], trq[64:128, :])


        # ----- a_g^T via matmul (gate col as lhsT), sigmoid on partition 0 ---
        for h in range(G):
            for blk in range(4):
                agz_ps = psm.tile([1, 512], F32, tag="mm")
                nc.tensor.matmul(
                    agz_ps[:],
                    wqkvg_sb[:, 0, 385 + h : 386 + h],
                    xT_sb[:, 0, 512 * blk : 512 * blk + 512],
                    start=True, stop=True)
                nc.scalar.activation(agS_sb[0:1, h, 512 * blk : 512 * blk + 512],
                                     agz_ps[:], ACTF.Sigmoid)

        # ---------------- phase 2: attention (512-wide q tiles) ----------
        for c in range(4):
            for h in range(G):
                at_ps = psa.tile([128, 512], F32, tag="acc")
                nkt = 4 * c + 4
                for kt in range(nkt):
                    # valid q range within this 512-q tile (causal)
                    i = kt - 4 * c
                    v0 = 128 * i if i > 0 else 0     # first valid q col
                    sc_ps = psm.tile([128, 512], F32, tag="mm")
                    nc.tensor.matmul(
                        sc_ps[:, v0:512], kT_sb[:, 128 * kt : 128 * kt + 128],
                        qT_sb[:, h, 512 * c + v0 : 512 * c + 512],
                        start=True, stop=True)
                    pT = ptp.tile([128, 512], BF16, tag="pT")
                    nc.scalar.activation(pT[:, v0:512], sc_ps[:, v0:512],
                                         ACTF.Exp, scale=SCALE)
                    if i >= 0:
                        # diagonal 128x128 block at q cols [v0, v0+128)
                        nc.vector.tensor_mul(
                            pT[:, v0 : v0 + 128], pT[:, v0 : v0 + 128],
                            maskd_sb[:, 128 * kt : 128 * kt + 128])
                    nc.tensor.matmul(
                        at_ps[0:65, v0:512], vaug_sb[:, kt, :], pT[:, v0:512],
                        start=(kt == 0), stop=(kt == nkt - 1))

                # epilogue: s_vec = 2*sigmoid(ag)/Z broadcast over dims
                rz = ep.tile([1, 512], F32, tag="rz")
                nc.vector.reciprocal(rz[:], at_ps[64:65, :])
                sv = ep.tile([1, 512], F32, tag="sv")
                nc.vector.tensor_mul(sv[:], rz[:],
                                     agS_sb[0:1, h, 512 * c : 512 * c + 512])
                bc = ep.tile([64, 512], F32, tag="bc")
                nc.gpsimd.partition_broadcast(bc[:], sv[:])
                po = 64 * (h % 2)
                nc.vector.tensor_mul(
                    agin_sb[po : po + 64, h // 2, 512 * c : 512 * c + 512],
                    at_ps[0:64, :], bc[:])

        # ---------------- phase 2.5: AllToAll (8-rank; shards duplicated to
        # both batch halves, receiver selects via zero-padded wo) ----------
        a2a_in = dram.tile([2048, 512], BF16)
        a2a_out = dram.tile([2048, 512], BF16)
        for b2 in range(2):
            for p2 in range(4):
                r0 = 1024 * b2 + 256 * p2
                nc.sync.dma_start(
                    out=a2a_in[r0 : r0 + 256].rearrange("(i p) q -> p i q", p=128),
                    in_=agin_sb[:, :, 512 * p2 : 512 * p2 + 512])
        nc.gpsimd.collective_compute(
            "AllToAll", ALU.bypass, replica_groups=replica_groups,
            ins=[a2a_in[:]], outs=[a2a_out[:]])
        nc.sync.dma_start(out=oT_sb[:],
                          in_=a2a_out[:].rearrange("(t p) q -> p t q", p=128))

        # ---------------- phase 3: o-projection ----------------
        for qs in range(4):
            for hf in range(2):
                op_ps = psa.tile([128, 512], F32, tag="acc")
                for r in range(16):
                    nc.tensor.matmul(
                        op_ps[:], oT_sb[:, r, 128 * qs : 128 * qs + 128],
                        wo_sb[:, r, 512 * hf : 512 * hf + 512],
                        start=(r == 0), stop=(r == 15))
                o_t = work.tile([128, 512], F32, tag="ot")
                nc.vector.tensor_copy(o_t[:], op_ps[:])
                nc.sync.dma_start(
                    out=out[128 * qs : 128 * qs + 128, 512 * hf : 512 * hf + 512],
                    in_=o_t[:])
    nc.finalize()
    return nc


def make_in_maps(x, token_ids, mask, w_q, w_k, w_v, w_o, ve_embed,
                 value_lambda, ve_lambda, ve_gate, attn_gate):
    x = np.asarray(x, np.float32)
    token_ids = np.asarray(token_ids)
    mask = np.asarray(mask)
    w_q = np.asarray(w_q, np.float32)
    w_k = np.asarray(w_k, np.float32)
    w_v = np.asarray(w_v, np.float32)
    w_o = np.asarray(w_o, np.float32)
    ve_embed = np.asarray(ve_embed, np.float32)
    ve_gate = np.asarray(ve_gate, np.float32)
    attn_gate = np.asarray(attn_gate, np.float32)
    lam = np.array([[float(value_lambda), float(ve_lambda)]], np.float32)

    half = ROT // 2
    inv_freq = 1.0 / (1024.0 ** (np.arange(half, dtype=np.float32) / half))
    ang = np.arange(S, dtype=np.float32)[:, None] * inv_freq[None, :]
    cos = np.cos(ang).astype(np.float32)          # [S, 16]
    sin = np.sin(ang).astype(np.float32)
    trig = np.concatenate([cos.reshape(ST, 128, HALF), sin.reshape(ST, 128, HALF)],
                          axis=2)                  # [ST, 128, 32]
    trig = np.ascontiguousarray(trig.transpose(1, 0, 2).reshape(128, ST * 2 * HALF))

    maskd = np.empty((128, S), np.float32)
    for t in range(ST):
        blk = mask[128 * t : 128 * t + 128, 128 * t : 128 * t + 128]
        maskd[:, 128 * t : 128 * t + 128] = blk.T.astype(np.float32)

    wo_pad = np.zeros((2, 2 * D, D), np.float32)
    for b in range(2):
        wo_pad[b, 1024 * b : 1024 * b + 1024] = w_o
    in_maps = []
    for c in range(8):
        b, m = c // 4, c % 4
        tok = np.ascontiguousarray(
            token_ids[b].astype(np.int32).reshape(ST, 128).T)   # [128, ST]
        kvgm = np.zeros((D, 389), np.float32)
        kvgm[:, 0:256] = w_q[:, 256 * m : 256 * m + 256]
        kvgm[:, 256:320] = w_k[:, 64 * m : 64 * m + 64]
        kvgm[:, 320:384] = w_v[:, 64 * m : 64 * m + 64]
        kvgm[0:GIN, 384] = ve_gate[:, m]
        kvgm[0:GIN, 385:389] = attn_gate[:, 4 * m : 4 * m + 4]
        in_maps.append({
            "xT": np.ascontiguousarray(x[b].T),
            "wqkvg": kvgm,
            "wo": wo_pad[b],
            "lam": lam,
            "vemb": np.ascontiguousarray(ve_embed[:, 64 * m : 64 * m + 64]),
            "tok": tok,
            "trig": trig,
            "maskd": maskd,
        })
    return in_maps


def unshard(results):
    out = np.empty((B, S, D), np.float32)
    for c in range(8):
        b, m = c // 4, c % 4
        out[b, 512 * m : 512 * m + 512, :] = results[c]["out"]
    return out


_NC_CACHE = {}


def kernel(**inputs):
    from concourse.bass_utils import run_bass_kernel_spmd
    if "nc" not in _NC_CACHE:
        _NC_CACHE["nc"] = build_nc()
    nc = _NC_CACHE["nc"]
    in_maps = make_in_maps(**inputs)
    res = run_bass_kernel_spmd(nc, in_maps, core_ids=list(range(8)))
    return unshard(res.results)


if __name__ == "__main__":
    pass
